# revision 11
# baseline (speedup 1.0000x reference)
"""Trainium2 Bass kernel for nn_CONV_COMPLEX_v1 (complex CNN + FC heads).

Self-contained: builds all Toeplitz/block-diag weights in numpy, compiles one
SPMD Bass/Tile program, shards batch over 8 cores, gathers [4096, 3] output.

v2 restructure vs baseline:
 - all conv/FC biases folded into the PSUM->SBUF bf16 copies (scalar ACT bias
   column APs); bias matmuls eliminated. RECSEL reads the biased bf16 im.
 - branch1 (conv32) cleaky batched across chunk-pairs into 2 [128,1024] calls.
 - conv1/conv2 even/odd wide pairs merged into single [128, 2048] cleaky ops.
 - conv3 pool output + branch1 output written directly into fcin tiles (fball
   staging eliminated).
 - XN build + pool-tree maxes on gpsimd; partition-shift copies via DMA.
"""
import sys
if '/opt/trn_rl_repo' not in sys.path:
    sys.path.insert(0, '/opt/trn_rl_repo')

import numpy as np
import ml_dtypes
import concourse.bass as bass
import concourse.bacc as bacc
import concourse.mybir as mybir
from concourse import tile, dve_ops
from concourse.dve_spec import Spec, Src0, Src1, C0, C1, select
from concourse.bass_utils import run_bass_kernel_spmd

AF = mybir.ActivationFunctionType
OP = mybir.AluOpType
dt = mybir.dt
F32 = dt.float32
BF16 = dt.bfloat16

SLOPE = 0.05
B, T, C = 4096, 64, 14
NCORES = 8
BC = B // NCORES          # 512 batch per core
R = BC * 14               # 7168 rows per core
CH = 512                  # row chunk
NCHUNK = R // CH          # 14
BIG = 1e30

# ---------------------------------------------------------------- custom op
def _register(op):
    if op.name in dve_ops._SUB_OPCODE_FOR_NAME:
        return
    dve_ops.OPS.append(op)
    dve_ops._SUB_OPCODE_FOR_NAME[op.name] = max(dve_ops._SUB_OPCODE_FOR_NAME.values()) + 1
    dve_ops.CUSTOM_DVE_SPECS[op.name] = op.spec


from concourse.dve_spec import AluOp, Bin, Zero, C2

_not_im = Bin(AluOp.BITWISE_NOT, Src0, Src0)
_y0 = _not_im * C0
_y1 = _y0 * (C1 - Src0 * _y0)


def _recsel_ref(in0, in1, s0, s1, imm2):
    nx = (~in0.view(np.int32)).view(np.float32)
    y0 = nx * s0
    y1 = y0 * (s1 - in0 * y0)
    return np.where(in0 < 0, (in1 * y1).astype(np.float32),
                    np.float32(imm2)).astype(np.float32)


# g = select(im < 0, re * recip1nr(im), -BIG); im=Src0, re=Src1
CLEAKY_RECSEL = dve_ops.DveOp(
    "CLEAKY_RECSEL",
    Spec(body=select(Src0 < Zero, Src1 * _y1, C2), reference=_recsel_ref),
    subdim=False,
    uops_sha={"v3": "222e6a151211d0d0", "v4": "17208ec156568f9e"},
)
CLEAKY_SELMUL = dve_ops.DveOp(
    "CLEAKY_SELMUL",
    Spec(body=select(Src1 < C0, Src0 * Src1, C1),
         reference=lambda in0, in1, s0, s1, imm2: np.where(
             in1 < s0, (in0 * in1).astype(np.float32), np.float32(s1)).astype(np.float32)),
    subdim=False,
    uops_sha={"v3": "7defef3488ac79a9", "v4": "1911aa2b7a375206"},
)
_register(CLEAKY_SELMUL)
_register(CLEAKY_RECSEL)
RECSEL_S0 = -0.23549792
RECSEL_S1 = 2.0017324

# ---------------------------------------------------------------- weights (numpy)
GROUPS = [
    (0, 3), (1, 4), (2, 5), (None, 6),
    (0, 7), (1, 8), (2, 9), (None, 10),
    ((0, 1), 13), ((1, 2), 13), ((2, 0), 13),
    ((0, 1), 11), ((1, 2), 12), ((2, 0), 13),
]


def _xn_row(comp, t):
    if t < 32:
        return comp * 32 + t
    return 64 + comp * 32 + (t - 32)


# bias column index map for BCOL [128, 24]
BC_C1RE0, BC_C1RE1, BC_C1IM0, BC_C1IM1 = 0, 1, 2, 3
BC_C2RE, BC_C2IM, BC_C3RE, BC_C3IM = 4, 5, 6, 7
BC_B1RE, BC_B1IM = 8, 9
BC_FC1RE, BC_FC1IM = 10, 11
BC_FC2RE0, BC_FC2RE1, BC_FC2RE2, BC_FC2RE3 = 12, 13, 14, 15
BC_FC2IM0, BC_FC2IM1, BC_FC2IM2, BC_FC2IM3 = 16, 17, 18, 19
BC_FC3RE, BC_FC3IM = 20, 21
BC_FC4RE, BC_FC4IM, BC_FC5 = 22, 23, 24   # FC4 biases on rows 0:60; FC5 rows 0:3
NBCOL = 25


def _build_weights(inp):
    w = {}
    # conv32 (branch1): W32 [129, 64]: cols 0:32 re-out (co*3+p, pad 24:32), 32:64 im-out
    wr, wi = inp['conv32_w'].real, inp['conv32_w'].imag
    W32 = np.zeros((129, 64), np.float32)
    for co in range(8):
        for p in range(3):
            m = co * 3 + p
            for k in range(32):
                t = 16 * p + k
                W32[_xn_row(0, t), m] += wr[co, 0, k]
                W32[_xn_row(1, t), m] += -wi[co, 0, k]
                W32[_xn_row(0, t), 32 + m] += wi[co, 0, k]
                W32[_xn_row(1, t), 32 + m] += wr[co, 0, k]
    w['W32'] = W32.astype(ml_dtypes.bfloat16)

    # conv1: W1 [65, 1024], tile q cols q*128..; q = parity*4 + comp_o*2 + cihalf
    wr, wi = inp['c3w1'].real, inp['c3w1'].imag
    W1 = np.zeros((65, 1024), np.float32)
    for parity in range(2):
        for comp_o in range(2):
            for cihalf in range(2):
                q = parity * 4 + comp_o * 2 + cihalf
                for ci_loc in range(8):
                    ci = cihalf * 8 + ci_loc
                    for l2 in range(16):
                        l = 2 * l2 + parity
                        m = q * 128 + ci_loc * 16 + l2
                        for k in range(3):
                            tp = (l + k - 1) % 32
                            if comp_o == 0:
                                W1[tp, m] += wr[ci, 0, k]
                                W1[32 + tp, m] += -wi[ci, 0, k]
                            else:
                                W1[tp, m] += wi[ci, 0, k]
                                W1[32 + tp, m] += wr[ci, 0, k]
    w['W1'] = W1.astype(ml_dtypes.bfloat16)

    # conv2: W2 [4, 128, 1024] kc-major
    wr, wi = inp['c3w2'].real, inp['c3w2'].imag
    W2 = np.zeros((4, 128, 1024), np.float32)
    for comp_o in range(2):
        for parity in range(2):
            for l3half in range(2):
                mt = comp_o * 4 + parity * 2 + l3half
                for l3_loc in range(4):
                    l = 2 * (l3half * 4 + l3_loc) + parity
                    for co in range(32):
                        m = mt * 128 + l3_loc * 32 + co
                        for ci in range(16):
                            for k in range(3):
                                li = (l + k - 1) % 16
                                krow = (ci % 8) * 16 + li
                                if comp_o == 0:
                                    W2[0 + ci // 8, krow, m] += wr[co, ci, k]
                                    W2[2 + ci // 8, krow, m] += -wi[co, ci, k]
                                else:
                                    W2[0 + ci // 8, krow, m] += wi[co, ci, k]
                                    W2[2 + ci // 8, krow, m] += wr[co, ci, k]
    w['W2'] = W2.astype(ml_dtypes.bfloat16)

    # conv3: W3 [4, 128, 512]; M tiles: mt = comp_o*2 + lhalf, part = l_loc*32+co
    wr, wi = inp['c3w3'].real, inp['c3w3'].imag
    W3 = np.zeros((4, 128, 512), np.float32)
    for comp_o in range(2):
        for lhalf in range(2):
            mt = comp_o * 2 + lhalf
            for l_loc in range(4):
                l = lhalf * 4 + l_loc
                for co in range(32):
                    m = mt * 128 + l_loc * 32 + co
                    for ci in range(32):
                        for k in range(3):
                            j = (l + k - 1) % 8
                            kc = 0 if j < 4 else 1
                            krow = (j % 4) * 32 + ci
                            if comp_o == 0:
                                W3[0 + kc, krow, m] += wr[co, ci, k]
                                W3[2 + kc, krow, m] += -wi[co, ci, k]
                            else:
                                W3[0 + kc, krow, m] += wi[co, ci, k]
                                W3[2 + kc, krow, m] += wr[co, ci, k]
    w['W3'] = W3.astype(ml_dtypes.bfloat16)

    # FC1: [14, 128, 256]; K idx = ch*64 + comp*32 + f (blocks of 7 tiles each)
    wr, wi = inp['hw1'].real, inp['hw1'].imag
    FC1 = np.zeros((14, 128, 256), np.float32)
    for h in range(3):
        for o in range(40):
            m = h * 40 + o
            for f in range(56):
                for ch in range(14):
                    i = f * 14 + ch
                    if f < 24:
                        base, fl = 0, f
                    else:
                        base, fl = 7, f - 24
                    kr = ch * 64 + 0 * 32 + fl
                    ki = ch * 64 + 1 * 32 + fl
                    FC1[base + kr // 128, kr % 128, m] += wr[h, o, i]
                    FC1[base + ki // 128, ki % 128, m] += -wi[h, o, i]
                    FC1[base + kr // 128, kr % 128, 128 + m] += wi[h, o, i]
                    FC1[base + ki // 128, ki % 128, 128 + m] += wr[h, o, i]
    w['FC1'] = FC1.astype(ml_dtypes.bfloat16)

    # FC2: [2, 128, 1024] (k-tile0 from z1re, 1 from z1im)
    wr, wi = inp['hw2'].real, inp['hw2'].imag
    FC2 = np.zeros((2, 128, 1024), np.float32)
    for h in range(3):
        for o in range(160):
            m = h * 160 + o
            for i in range(40):
                k = h * 40 + i
                FC2[0, k, m] += wr[h, o, i]
                FC2[1, k, m] += -wi[h, o, i]
                FC2[0, k, 512 + m] += wi[h, o, i]
                FC2[1, k, 512 + m] += wr[h, o, i]
    w['FC2'] = FC2.astype(ml_dtypes.bfloat16)

    # FC3: [8, 128, 256] (tiles 0..3 = z2re rows, 4..7 = z2im)
    wr, wi = inp['hw3'].real, inp['hw3'].imag
    FC3 = np.zeros((8, 128, 256), np.float32)
    for h in range(3):
        for o in range(40):
            m = h * 40 + o
            for i in range(160):
                k = h * 160 + i
                FC3[k // 128, k % 128, m] += wr[h, o, i]
                FC3[4 + k // 128, k % 128, m] += -wi[h, o, i]
                FC3[k // 128, k % 128, 128 + m] += wi[h, o, i]
                FC3[4 + k // 128, k % 128, 128 + m] += wr[h, o, i]
    w['FC3'] = FC3.astype(ml_dtypes.bfloat16)

    # FC4: [2, 128, 128] (tile0 z3re, tile1 z3im); M: re 0:64 (60+4pad), im 64:128
    wr, wi = inp['hw4'].real, inp['hw4'].imag
    FC4 = np.zeros((2, 128, 128), np.float32)
    for h in range(3):
        for o in range(20):
            m = h * 20 + o
            for i in range(40):
                k = h * 40 + i
                FC4[0, k, m] += wr[h, o, i]
                FC4[1, k, m] += -wi[h, o, i]
                FC4[0, k, 64 + m] += wi[h, o, i]
                FC4[1, k, 64 + m] += wr[h, o, i]
    w['FC4'] = FC4.astype(ml_dtypes.bfloat16)

    # FC5: [128, 3] (rows 0:64 z4re, 64:128 z4im)
    wr, wi = inp['hw5'].real, inp['hw5'].imag
    FC5 = np.zeros((128, 3), np.float32)
    for h in range(3):
        for i in range(20):
            k = h * 20 + i
            FC5[k, h] += wr[h, 0, i]
            FC5[64 + k, h] += -wi[h, 0, i]
    w['FC5'] = FC5.astype(ml_dtypes.bfloat16)

    # BCOL [128, NBCOL] f32: per-partition bias columns, folded into the
    # PSUM->SBUF copies inside cleaky (no bias matmuls anywhere).
    BCOL = np.zeros((128, NBCOL), np.float32)
    b1r, b1i = inp['c3b1'].real, inp['c3b1'].imag
    for p in range(128):
        BCOL[p, BC_C1RE0] = b1r[p // 16]
        BCOL[p, BC_C1RE1] = b1r[8 + p // 16]
        BCOL[p, BC_C1IM0] = b1i[p // 16]
        BCOL[p, BC_C1IM1] = b1i[8 + p // 16]
        BCOL[p, BC_C2RE] = inp['c3b2'].real[p % 32]
        BCOL[p, BC_C2IM] = inp['c3b2'].imag[p % 32]
        BCOL[p, BC_C3RE] = inp['c3b3'].real[p % 32]
        BCOL[p, BC_C3IM] = inp['c3b3'].imag[p % 32]
        m = p % 32
        if m < 24:
            BCOL[p, BC_B1RE] = inp['conv32_b'].real[m // 3]
            BCOL[p, BC_B1IM] = inp['conv32_b'].imag[m // 3]
    hb1, hb2, hb3, hb4, hb5 = (inp[f'hb{i}'] for i in range(1, 6))
    for h in range(3):
        for o in range(40):
            BCOL[h * 40 + o, BC_FC1RE] = hb1.real[h, o]
            BCOL[h * 40 + o, BC_FC1IM] = hb1.imag[h, o]
            BCOL[h * 40 + o, BC_FC3RE] = hb3.real[h, o]
            BCOL[h * 40 + o, BC_FC3IM] = hb3.imag[h, o]
        for o in range(160):
            m = h * 160 + o
            BCOL[m % 128, BC_FC2RE0 + m // 128] = hb2.real[h, o]
            BCOL[m % 128, BC_FC2IM0 + m // 128] = hb2.imag[h, o]
        for o in range(20):
            BCOL[h * 20 + o, BC_FC4RE] = hb4.real[h, o]
            BCOL[h * 20 + o, BC_FC4IM] = hb4.imag[h, o]
        BCOL[h, BC_FC5] = hb5.real[h, 0]
    w['BCOL'] = BCOL
    return w


# ---------------------------------------------------------------- bass program
_CACHE = {}


def _build_program(repeat=1):
    nc = bacc.Bacc("TRN2", target_bir_lowering=False, debug=False, num_devices=NCORES)

    x_d = nc.dram_tensor("x", [BC, T, C], F32, kind="ExternalInput").ap()
    wd = {}
    for name, shape, dtp in [
        ('W32', [129, 64], BF16), ('W1', [65, 1024], BF16),
        ('W2', [4, 128, 1024], BF16),
        ('W3', [4, 128, 512], BF16),
        ('FC1', [14, 128, 256], BF16), ('FC2', [2, 128, 1024], BF16),
        ('FC3', [8, 128, 256], BF16),
        ('FC4', [2, 128, 128], BF16),
        ('FC5', [128, 3], BF16),
        ('BCOL', [128, NBCOL], F32),
    ]:
        wd[name] = nc.dram_tensor(name, shape, dtp, kind="ExternalInput").ap()
    out_d = nc.dram_tensor("out", [BC, 3], F32, kind="ExternalOutput").ap()

    with tile.TileContext(nc) as tc:
        for _ in range(repeat):
            _emit(nc, tc, x_d, wd, out_d)

    nc.compile()
    return nc


def _emit(nc, tc, x_d, wd, out_d):
    import contextlib

    with contextlib.ExitStack() as gctx:
        cpool = gctx.enter_context(tc.tile_pool(name="consts", bufs=1))
        ps = gctx.enter_context(tc.tile_pool(name="ps", bufs=1, space="PSUM"))

        bias_s = cpool.tile([128, 1], F32, tag="bias_s", name="bias_s")
        bias_c = cpool.tile([128, 1], F32, tag="bias_c", name="bias_c")
        nc.gpsimd.memset(bias_s[:], 0.475 * np.pi)
        nc.gpsimd.memset(bias_c[:], 0.025 * np.pi)
        bcol = cpool.tile([128, NBCOL], F32, tag="bcol", name="bcol")
        nc.sync.dma_start(bcol[:], wd['BCOL'][:])

        def bc(col, p0=0, p1=128):
            return bcol[p0:p1, col:col + 1]

        # FC weight pool; DMAs emitted after the XN build x0 DMA.
        fwp = gctx.enter_context(tc.tile_pool(name="fw", bufs=1))
        fwt = {}

        def load_fc_weights():
            for name in ('FC1', 'FC2', 'FC3', 'FC4'):
                ap = wd[name]
                shape = ap.shape
                tiles = []
                for i in range(shape[0]):
                    t = fwp.tile(list(shape[1:]), ap.dtype, tag=f"w_{name}_{i}",
                                 name=f"w_{name}_{i}")
                    nc.sync.dma_start(t[:], ap[i])
                    tiles.append(t)
                fwt[name] = tiles
            fwt['fc5a'] = fwp.tile([64, 3], BF16, tag="fc5a", name="fc5a")
            fwt['fc5b'] = fwp.tile([64, 3], BF16, tag="fc5b", name="fc5b")
            nc.sync.dma_start(fwt['fc5a'][:], wd['FC5'][0:64, :])
            nc.sync.dma_start(fwt['fc5b'][:], wd['FC5'][64:128, :])

        # work pool for cleaky intermediates: single [128, 2048] variant,
        # sliced per call.
        work = gctx.enter_context(tc.tile_pool(name="cwork", bufs=1))
        WN = 2048

        def cleaky_multi(parts, P=128, ntot=None, out_re=None, out_im=None,
                         pre_re=None, pre_im=None, bufs=2):
            """parts: list of (re_ap, im_ap, n, re_biases, im_biases) where
            *_biases is a list of (bias_col_ap_or_None, width) covering n.
            pre_re/pre_im: already-assembled bf16 SBUF tiles (skip copies)."""
            if ntot is None:
                ntot = sum(p[2] for p in parts) if parts else pre_re.shape[-1]
            if pre_re is not None:
                re_bf, im_bf = pre_re, pre_im
            else:
                re_t = work.tile([128, WN], BF16, tag="ck_rebf", name="ck_rebf", bufs=bufs)
                im_t = work.tile([128, WN], BF16, tag="ck_imbf", name="ck_imbf", bufs=bufs)
                re_bf = re_t[0:P, 0:ntot]
                im_bf = im_t[0:P, 0:ntot]
                off = 0
                for re_ap, im_ap, n, rbs, ibs in parts:
                    for dst, src, bs in ((re_bf, re_ap, rbs), (im_bf, im_ap, ibs)):
                        o2 = 0
                        for bias, wdt in bs:
                            if bias is None:
                                nc.scalar.copy(dst[:, off + o2:off + o2 + wdt],
                                               src[:, o2:o2 + wdt])
                            else:
                                nc.scalar.activation(dst[:, off + o2:off + o2 + wdt],
                                                     src[:, o2:o2 + wdt], AF.Identity,
                                                     bias=bias, scale=1.0)
                            o2 += wdt
                    off += n
            g_t = work.tile([128, WN], F32, tag="ck_g", name="ck_g", bufs=bufs)
            g = g_t[0:P, 0:ntot]
            nc.vector._custom_dve(CLEAKY_RECSEL, out=g, in0=im_bf, in1=re_bf,
                                  s0=RECSEL_S0, s1=RECSEL_S1, imm2=-BIG)
            b_t = work.tile([128, WN], F32, tag="ck_b", name="ck_b", bufs=bufs)
            b = b_t[0:P, 0:ntot]
            nc.scalar.activation(b, g, AF.Arctan)
            s_t = work.tile([128, WN], BF16, tag="ck_s", name="ck_s", bufs=bufs)
            c_t = work.tile([128, WN], BF16, tag="ck_c", name="ck_c", bufs=bufs)
            s = s_t[0:P, 0:ntot]
            c = c_t[0:P, 0:ntot]
            nc.scalar.activation(s, b, AF.Sin, scale=0.95, bias=bias_s[0:P, :])
            nc.scalar.activation(c, b, AF.Sin, scale=-0.95, bias=bias_c[0:P, :])
            sc1 = work.tile([128, WN], BF16, tag="ck_sc1", name="ck_sc1", bufs=bufs)
            sc2 = work.tile([128, WN], BF16, tag="ck_sc2", name="ck_sc2", bufs=bufs)
            p1 = sc1[0:P, 0:ntot]
            q1 = sc2[0:P, 0:ntot]
            if out_re is None:
                ore_t = work.tile([128, WN], BF16, tag="ck_ore", name="ck_ore", bufs=bufs)
                out_re = ore_t[0:P, 0:ntot]
            if out_im is None:
                oim_t = work.tile([128, WN], BF16, tag="ck_oim", name="ck_oim", bufs=bufs)
                out_im = oim_t[0:P, 0:ntot]
            nc.vector.tensor_tensor(out=p1, in0=re_bf, in1=c, op=OP.mult)
            nc.vector.tensor_tensor(out=q1, in0=im_bf, in1=s, op=OP.mult)
            nc.vector.tensor_tensor(out=out_re, in0=p1, in1=q1, op=OP.subtract)
            sc1b = work.tile([128, WN], BF16, tag="ck_sc1", name="ck_sc1", bufs=bufs)
            sc2b = work.tile([128, WN], BF16, tag="ck_sc2", name="ck_sc2", bufs=bufs)
            p2 = sc1b[0:P, 0:ntot]
            q2 = sc2b[0:P, 0:ntot]
            nc.vector.tensor_tensor(out=p2, in0=re_bf, in1=s, op=OP.mult)
            nc.vector.tensor_tensor(out=q2, in0=im_bf, in1=c, op=OP.mult)
            nc.vector.tensor_tensor(out=out_im, in0=p2, in1=q2, op=OP.add)
            return out_re, out_im

        # fcin tiles [128, BC] (K layout: ch*64 + comp*32 + f; tiles 0-6 branch1
        # feats f<24, tiles 7-13 conv3 feats). Written directly by the conv loop.
        fcp = gctx.enter_context(tc.tile_pool(name="fcp", bufs=1))
        fcin = [fcp.tile([128, BC], BF16, tag=f"fcin{t}", name=f"fcin{t}")
                for t in range(14)]
        for t in range(14):
            nc.gpsimd.memset(fcin[t][:], 0.0)

        # ================= conv phase =================
        with contextlib.ExitStack() as cctx:
            big = cctx.enter_context(tc.tile_pool(name="big", bufs=1))

            # XN build (two 64-partition tiles: lo = t 0..31, hi = t 32..63)
            xnlo = big.tile([64, R], BF16, tag="xnlo", name="xnlo")
            xnhi = big.tile([64, R], BF16, tag="xnhi", name="xnhi")
            with tc.tile_pool(name="x0pool", bufs=1) as x0pool:
                x0 = x0pool.tile([64, R], F32, tag="x0", name="x0")
                # split the transpose-gather DMA so the lo-half build can start
                # while the hi half is still loading
                for t0 in (0, 32):
                    nc.sync.dma_start(
                        x0[t0:t0 + 32].rearrange("t (b c) -> t b c", c=14),
                        x_d[:, t0:t0 + 32, :].rearrange("b t c -> t b c"))

                def xplane(c, lo):
                    return x0[lo:lo + 32].rearrange("t (b c) -> t b c", c=14)[:, :, c]

                # ch-major free dim: column r = ch*BC + b
                for ch, (respec, imc) in enumerate(GROUPS):
                    for lo, xt_ in ((0, xnlo), (32, xnhi)):
                        dst = xt_.rearrange("p (c b) -> p c b", b=BC)
                        if respec is None:
                            nc.vector.memset(dst[0:32, ch, :], 0.0)
                        elif isinstance(respec, tuple):
                            nc.any.tensor_tensor(out=dst[0:32, ch, :],
                                                 in0=xplane(respec[0], lo),
                                                 in1=xplane(respec[1], lo),
                                                 op=OP.subtract)
                        else:
                            nc.any.tensor_copy(dst[0:32, ch, :], xplane(respec, lo))
                        nc.any.tensor_copy(dst[32:64, ch, :], xplane(imc, lo))

            wpool = cctx.enter_context(tc.tile_pool(name="cw", bufs=1))
            x2p = cctx.enter_context(tc.tile_pool(name="x2p", bufs=2))
            b1p = cctx.enter_context(tc.tile_pool(name="b1p", bufs=2))

            # conv-phase weights
            wt = {}
            for name in ('W2', 'W3'):
                ap = wd[name]
                tiles = []
                for i in range(ap.shape[0]):
                    t = wpool.tile(list(ap.shape[1:]), ap.dtype, tag=f"w_{name}_{i}",
                                   name=f"w_{name}_{i}")
                    nc.sync.dma_start(t[:], ap[i])
                    tiles.append(t)
                wt[name] = tiles
            w32lo = wpool.tile([64, 64], BF16, tag="w32lo", name="w32lo")
            w32hi = wpool.tile([64, 64], BF16, tag="w32hi", name="w32hi")
            nc.sync.dma_start(w32lo[:], wd['W32'][0:64, :])
            nc.sync.dma_start(w32hi[:], wd['W32'][64:128, :])
            w1a = wpool.tile([64, 1024], BF16, tag="w1a", name="w1a")
            nc.sync.dma_start(w1a[:], wd['W1'][0:64, :])
            load_fc_weights()

            # branch1 batched state: asm tiles hold biased bf16 copies of the
            # conv32 psum for 4 (round A) / 3 (round B) chunk-pairs.
            b1_asm = [None, None]

            def b1_round(round_idx, npairs):
                # batched cleaky over [32*npairs, 1024] + direct fcin writes
                P = 32 * npairs
                ore, oim = cleaky_multi([], P=P, ntot=2 * CH,
                                        pre_re=b1_asm[0][0:P, :],
                                        pre_im=b1_asm[1][0:P, :])
                for q in range(npairs):
                    p = round_idx * 4 + q
                    for half in range(2):
                        hs = slice(half * CH, (half + 1) * CH)
                        kp = (half) * 64
                        nc.sync.dma_start(fcin[p][kp:kp + 24, :],
                                          ore[32 * q:32 * q + 24, hs])
                        nc.sync.dma_start(fcin[p][kp + 32:kp + 56, :],
                                          oim[32 * q:32 * q + 24, hs])

            for cidx in range(NCHUNK):
                cs = slice(cidx * CH, (cidx + 1) * CH)
                rhs_lo = xnlo[:, cs]
                rhs_hi = xnhi[:, cs]

                # --- branch1 conv32 matmuls at odd chunks; copies into asm
                if cidx % 2 == 1:
                    pair = (cidx - 1) // 2
                    rnd, q = divmod(pair, 4)
                    if q == 0:
                        b1_asm[0] = b1p.tile([128, 2 * CH], BF16, tag="b1ra",
                                             name="b1ra")
                        b1_asm[1] = b1p.tile([128, 2 * CH], BF16, tag="b1ia",
                                             name="b1ia")
                    pb = ps.tile([128, 2 * CH], F32, tag="cps", name="cps", bufs=4)
                    b1wim = ps.tile([128, 2 * CH], F32, tag="cps", name="cps", bufs=4)
                    for half, cx in ((0, cidx - 1), (1, cidx)):
                        hs = slice(half * CH, (half + 1) * CH)
                        rl = xnlo[:, cx * CH:(cx + 1) * CH]
                        rh = xnhi[:, cx * CH:(cx + 1) * CH]
                        nc.tensor.matmul(pb[0:32, hs], w32lo[:, 0:32], rl, start=True, stop=False)
                        nc.tensor.matmul(pb[0:32, hs], w32hi[:, 0:32], rh, start=False, stop=True)
                        nc.tensor.matmul(b1wim[0:32, hs], w32lo[:, 32:64], rl, start=True, stop=False)
                        nc.tensor.matmul(b1wim[0:32, hs], w32hi[:, 32:64], rh, start=False, stop=True)
                    qs = slice(32 * q, 32 * q + 32)
                    nc.scalar.activation(b1_asm[0][qs, :], pb[0:32, :], AF.Identity,
                                         bias=bc(BC_B1RE, 32 * q, 32 * q + 32), scale=1.0)
                    nc.scalar.activation(b1_asm[1][qs, :], b1wim[0:32, :], AF.Identity,
                                         bias=bc(BC_B1IM, 32 * q, 32 * q + 32), scale=1.0)
                    if pair == 3:
                        b1_round(0, 4)
                    elif pair == 6:
                        b1_round(1, 3)

                # --- conv1: 4 wide psum tiles; merged e|o cleaky [128, 2048]
                def c1pair(q0, q1):
                    pt = ps.tile([128, 2 * CH], F32, tag="cps", name="cps", bufs=4)
                    for j, q in enumerate((q0, q1)):
                        sl = slice(j * CH, (j + 1) * CH)
                        nc.tensor.matmul(pt[:, sl], w1a[:, q * 128:(q + 1) * 128],
                                         rhs_hi, start=True, stop=True)
                    return pt

                e_re = c1pair(0, 1); e_im = c1pair(2, 3)
                o_re = c1pair(4, 5); o_im = c1pair(6, 7)
                c1rb = [(bc(BC_C1RE0), CH), (bc(BC_C1RE1), CH)]
                c1ib = [(bc(BC_C1IM0), CH), (bc(BC_C1IM1), CH)]
                or2, oi2 = cleaky_multi([
                    (e_re, e_im, 2 * CH, c1rb, c1ib),
                    (o_re, o_im, 2 * CH, c1rb, c1ib)])
                X2re = x2p.tile([128, 2 * CH], BF16, tag="x2re", name="x2re")
                X2im = x2p.tile([128, 2 * CH], BF16, tag="x2im", name="x2im")
                nc.vector.tensor_tensor(out=X2re[:], in0=or2[:, 0:2 * CH],
                                        in1=or2[:, 2 * CH:4 * CH], op=OP.max)
                nc.vector.tensor_tensor(out=X2im[:], in0=oi2[:, 0:2 * CH],
                                        in1=oi2[:, 2 * CH:4 * CH], op=OP.max)

                x2kc = [X2re[:, 0:CH], X2re[:, CH:2 * CH], X2im[:, 0:CH], X2im[:, CH:2 * CH]]

                # --- conv2: merged e|o cleaky
                def c2pair(m0, m1):
                    pt = ps.tile([128, 2 * CH], F32, tag="cps", name="cps", bufs=4)
                    for j, mt in enumerate((m0, m1)):
                        sl = slice(j * CH, (j + 1) * CH)
                        for kc in range(4):
                            nc.tensor.matmul(pt[:, sl], wt['W2'][kc][:, mt * 128:(mt + 1) * 128],
                                             x2kc[kc], start=(kc == 0), stop=(kc == 3))
                    return pt

                e_re = c2pair(0, 1); e_im = c2pair(4, 5)
                o_re = c2pair(2, 3); o_im = c2pair(6, 7)
                c2rb = [(bc(BC_C2RE), 2 * CH)]
                c2ib = [(bc(BC_C2IM), 2 * CH)]
                or2, oi2 = cleaky_multi([
                    (e_re, e_im, 2 * CH, c2rb, c2ib),
                    (o_re, o_im, 2 * CH, c2rb, c2ib)])
                X3re = x2p.tile([128, 2 * CH], BF16, tag="x3re", name="x3re")
                X3im = x2p.tile([128, 2 * CH], BF16, tag="x3im", name="x3im")
                nc.vector.tensor_tensor(out=X3re[:], in0=or2[:, 0:2 * CH],
                                        in1=or2[:, 2 * CH:4 * CH], op=OP.max)
                nc.vector.tensor_tensor(out=X3im[:], in0=oi2[:, 0:2 * CH],
                                        in1=oi2[:, 2 * CH:4 * CH], op=OP.max)

                x3kc = [X3re[:, 0:CH], X3re[:, CH:2 * CH], X3im[:, 0:CH], X3im[:, CH:2 * CH]]

                # --- conv3: one wide pair (R0,R1)/(I0,I1)
                def c3pair(m0, m1):
                    pt = ps.tile([128, 2 * CH], F32, tag="cps", name="cps", bufs=4)
                    for j, mt in enumerate((m0, m1)):
                        sl = slice(j * CH, (j + 1) * CH)
                        for kc in range(4):
                            nc.tensor.matmul(pt[:, sl], wt['W3'][kc][:, mt * 128:(mt + 1) * 128],
                                             x3kc[kc], start=(kc == 0), stop=(kc == 3))
                    return pt

                wre = c3pair(0, 1)
                wim = c3pair(2, 3)
                wore, woim = cleaky_multi([
                    (wre, wim, 2 * CH, [(bc(BC_C3RE), 2 * CH)], [(bc(BC_C3IM), 2 * CH)])])
                # pool tree: max over wide halves (gpsimd), partition shifts (DMA)
                kt, kpo = 7 + cidx // 2, (cidx % 2) * 64
                for srcw, fo in ((wore, 0), (woim, 32)):
                    G = work.tile([128, CH], BF16, tag="c3g", name="c3g", bufs=2)
                    nc.vector.tensor_tensor(out=G[:], in0=srcw[:, 0:CH],
                                            in1=srcw[:, CH:2 * CH], op=OP.max)
                    t64 = work.tile([64, CH], BF16, tag="c3t64", name="c3t64", bufs=2)
                    nc.sync.dma_start(t64[:], G[64:128, :])
                    H = work.tile([64, CH], BF16, tag="c3h", name="c3h", bufs=2)
                    nc.vector.tensor_tensor(out=H[:], in0=G[0:64, :], in1=t64[:], op=OP.max)
                    t32 = work.tile([32, CH], BF16, tag="c3t32", name="c3t32", bufs=2)
                    nc.sync.dma_start(t32[:], H[32:64, :])
                    Fx = work.tile([32, CH], BF16, tag="c3f", name="c3f", bufs=2)
                    nc.vector.tensor_tensor(out=Fx[:], in0=H[0:32, :], in1=t32[:], op=OP.max)
                    nc.sync.dma_start(fcin[kt][kpo + fo:kpo + fo + 32, :], Fx[:])

        # ================= FC phase =================
        with contextlib.ExitStack() as fctx:
            zp = fctx.enter_context(tc.tile_pool(name="zp", bufs=1))

            def fc_matmul(wtiles, ktiles, mts, Mp):
                outs = {}
                for mt in mts:
                    pt = ps.tile([128, 2 * CH], F32, tag="cps", name="cps", bufs=4)
                    for k, (wtile, ktile) in enumerate(zip(wtiles, ktiles)):
                        nc.tensor.matmul(pt[0:Mp, 0:BC], wtile[:, mt * Mp:(mt + 1) * Mp],
                                         ktile, start=(k == 0), stop=(k == len(ktiles) - 1))
                    outs[mt] = pt
                return outs

            # FC1 (14 K-tiles, bias folded into copies)
            z1ps = fc_matmul(fwt['FC1'], [t[:] for t in fcin], [0, 1], 128)
            z1re = zp.tile([128, BC], BF16, tag="z1re", name="z1re")
            z1im = zp.tile([128, BC], BF16, tag="z1im", name="z1im")
            cleaky_multi([(z1ps[0][0:128, 0:BC], z1ps[1][0:128, 0:BC], BC,
                           [(bc(BC_FC1RE), BC)], [(bc(BC_FC1IM), BC)])],
                         out_re=z1re[:], out_im=z1im[:])
            # FC2: wide pairs j=0,1 merged into one [128, 2048] cleaky
            parts = []
            for j in range(2):
                wre_t = ps.tile([128, 2 * CH], F32, tag="cps", name="cps", bufs=4)
                wim_t = ps.tile([128, 2 * CH], F32, tag="cps", name="cps", bufs=4)
                for k, mt in enumerate((2 * j, 2 * j + 1)):
                    sl = slice(k * BC, (k + 1) * BC)
                    for ki, (wtile, ktile) in enumerate(
                            [(fwt['FC2'][0], z1re[:]), (fwt['FC2'][1], z1im[:])]):
                        nc.tensor.matmul(wre_t[:, sl], wtile[:, mt * 128:(mt + 1) * 128],
                                         ktile, start=(ki == 0), stop=(ki == 1))
                    mti = 4 + mt
                    for ki, (wtile, ktile) in enumerate(
                            [(fwt['FC2'][0], z1re[:]), (fwt['FC2'][1], z1im[:])]):
                        nc.tensor.matmul(wim_t[:, sl], wtile[:, mti * 128:(mti + 1) * 128],
                                         ktile, start=(ki == 0), stop=(ki == 1))
                parts.append((wre_t, wim_t, 2 * BC,
                              [(bc(BC_FC2RE0 + 2 * j), BC), (bc(BC_FC2RE1 + 2 * j), BC)],
                              [(bc(BC_FC2IM0 + 2 * j), BC), (bc(BC_FC2IM1 + 2 * j), BC)]))
            # merged cleaky: out [128, 2048] = [mt0 | mt1 | mt2 | mt3] per batch
            orw, oiw = cleaky_multi(parts)
            z2k = [orw[:, 0:BC], orw[:, BC:2 * BC],
                   orw[:, 2 * BC:3 * BC], orw[:, 3 * BC:4 * BC],
                   oiw[:, 0:BC], oiw[:, BC:2 * BC],
                   oiw[:, 2 * BC:3 * BC], oiw[:, 3 * BC:4 * BC]]
            z3ps = fc_matmul(fwt['FC3'], z2k, [0, 1], 128)
            z3re = zp.tile([128, BC], BF16, tag="z3re", name="z3re")
            z3im = zp.tile([128, BC], BF16, tag="z3im", name="z3im")
            cleaky_multi([(z3ps[0][0:128, 0:BC], z3ps[1][0:128, 0:BC], BC,
                           [(bc(BC_FC3RE), BC)], [(bc(BC_FC3IM), BC)])],
                         out_re=z3re[:], out_im=z3im[:])
            # FC4: single psum [64, BC] re/im (M: re 0:64, im 64:128)
            z4ps = fc_matmul([fwt['FC4'][0], fwt['FC4'][1]],
                             [z3re[:], z3im[:]], [0, 1], 64)
            z4re = zp.tile([64, BC], BF16, tag="z4re", name="z4re")
            z4im = zp.tile([64, BC], BF16, tag="z4im", name="z4im")
            cleaky_multi([(z4ps[0][0:64, 0:BC], z4ps[1][0:64, 0:BC], BC,
                           [(bc(BC_FC4RE, 0, 64), BC)], [(bc(BC_FC4IM, 0, 64), BC)])],
                         P=64, out_re=z4re[:], out_im=z4im[:])
            # FC5 (re only, M=3); bias via the sigmoid activation
            p5 = ps.tile([128, 2 * CH], F32, tag="cps", name="cps", bufs=4)
            nc.tensor.matmul(p5[0:3, 0:BC], fwt['fc5a'][:], z4re[:], start=True, stop=False)
            nc.tensor.matmul(p5[0:3, 0:BC], fwt['fc5b'][:], z4im[:], start=False, stop=True)
            osb = zp.tile([3, BC], F32, tag="osb", name="osb")
            nc.scalar.activation(osb[:], p5[0:3, 0:BC], AF.Sigmoid,
                                 bias=bc(BC_FC5, 0, 3), scale=1.0)
            nc.sync.dma_start(out_d.rearrange("b h -> h b"), osb[:])


# ---------------------------------------------------------------- entry point
def kernel(**inputs):
    if 'nc' not in _CACHE:
        _CACHE['nc'] = _build_program()
    nc = _CACHE['nc']
    w = _build_weights(inputs)
    x = np.ascontiguousarray(inputs['x'], dtype=np.float32)
    in_maps = []
    for i in range(NCORES):
        m = {'x': x[i * BC:(i + 1) * BC]}
        m.update(w)
        in_maps.append(m)
    res = run_bass_kernel_spmd(nc, in_maps, list(range(NCORES)))
    out = np.concatenate([res.results[i]['out'] for i in range(NCORES)], axis=0)
    return out.astype(np.float32)


if __name__ == "__main__":
    d = np.load('/root/problem/ref_inputs.npz')
    inp = {k: d[k] for k in d.files}
    expected = np.load('/root/problem/ref_expected.npy')
    got = kernel(**inp)
    err = np.abs(got - expected)
    rel = np.linalg.norm(got - expected) / np.linalg.norm(expected)
    print("max abs err:", err.max(), " norm rel:", rel)
